# revision 8
# baseline (speedup 1.0000x reference)
"""Trainium2 Bass kernel for causal self-attention (B=4, T=2048, C=2048, H=16).

Sharding: 8 cores = 4 batches x 2 head-groups (8 heads each).

v2: fused per-head pipeline, no DRAM spills.
  A) v = x @ Wv             -> SBUF-resident bf16 [16 x (128, 1024)]
  Head loop h = 0..7:
    B_h) q_h^T, k_h^T = Wqk_h^T x^T + b, RoPE  -> SBUF bf16 [128, T]
    C_h) t-outer SDPA: scores, exp (ACT), causal mask, PV accumulate,
         denominator via ones-matmul, reciprocal_approx_fast -> y_h bf16
    The Tile scheduler overlaps C_h's ACT/DVE work with B_{h+1}'s matmuls.
  D) partial_out = y^T @ wp_rows -> [T, C] fp32 partial
Host sums core pairs per batch and adds b_proj.

All matmul operands bf16 (fp32 PSUM accumulate). x^T resident in SBUF
through phase A + all B_h as 64 [128,512] bf16 tiles.
"""

import contextlib
import sys

import numpy as np

sys.path.insert(0, "/opt/trn_rl_repo")

import ml_dtypes  # noqa: E402

import concourse.bass as bass  # noqa: E402,F401
import concourse.mybir as mybir  # noqa: E402
import concourse.tile as tile  # noqa: E402
from concourse import bacc  # noqa: E402

F32 = mybir.dt.float32
F32R = mybir.dt.float32r
BF16 = mybir.dt.bfloat16
F16 = mybir.dt.float16
AF = mybir.ActivationFunctionType
BF16NP = ml_dtypes.bfloat16

B, T, C = 4, 2048, 2048
H, D = 16, 128
HPC = 8            # heads per core
P = 128
NT = 512           # matmul moving free dim
TT = T // NT       # 4 token tiles
CC = C // P        # 16 contraction chunks over C
QK_CHUNKS = 2 * HPC
ROPE_BASE = 10000.0

_CACHE = {}


def _mm(nc, out, lhsT, rhs, **kw):
    nc.tensor.matmul(out, lhsT, rhs, **kw)


def build_program():
    nc = bacc.Bacc(name="csa_v2")

    xt = nc.dram_tensor("xt", (C, T), BF16, kind="ExternalInput")
    # wqk columns interleaved per head: head h owns cols [256h, 256h+256)
    # (first 128 = q scaled, next 128 = k)
    wqk = nc.dram_tensor("wqk", (C, QK_CHUNKS * P), BF16, kind="ExternalInput")
    bqk = nc.dram_tensor("bqk", (P, QK_CHUNKS), F32, kind="ExternalInput")
    wv = nc.dram_tensor("wv", (C, HPC * D), BF16, kind="ExternalInput")
    bvc = nc.dram_tensor("bvc", (P, HPC), F32, kind="ExternalInput")
    cs = nc.dram_tensor("cs", (P, T), F16, kind="ExternalInput")
    sw = nc.dram_tensor("sw", (P, T), F16, kind="ExternalInput")
    tri = nc.dram_tensor("tri", (P, P), BF16, kind="ExternalInput")
    onesm = nc.dram_tensor("onesm", (P, P), F32R, kind="ExternalInput")
    wp = nc.dram_tensor("wp", (HPC * D, C), BF16, kind="ExternalInput")
    out = nc.dram_tensor("out", (T, C), F32, kind="ExternalOutput")

    hd = D // 2

    with tile.TileContext(nc) as tc:
        es = contextlib.ExitStack()
        es2 = contextlib.ExitStack()
        with (
            tc.tile_pool(name="tabs", bufs=1) as tabs,
            tc.tile_pool(name="v_res", bufs=1) as v_res,
            tc.tile_pool(name="y_res", bufs=1) as y_res,
            tc.tile_pool(name="qk_res", bufs=1) as qk_res,
            tc.tile_pool(name="work", bufs=1) as work,
            tc.tile_pool(name="psum", bufs=1, space="PSUM") as psum,
        ):
            xt_res = es.enter_context(tc.tile_pool(name="xt_res", bufs=1))

            # ---- tables ----
            bqk_t = tabs.tile([P, QK_CHUNKS], F32, tag="bqk", name="bqk")
            nc.scalar.dma_start(bqk_t[:], bqk[:])
            cs_t = tabs.tile([P, T], F16, tag="cs", name="cs")
            nc.scalar.dma_start(cs_t[:], cs[:])
            sw_t = tabs.tile([P, T], F16, tag="sw", name="sw")
            nc.scalar.dma_start(sw_t[:], sw[:])
            tri_t = tabs.tile([P, P], BF16, tag="tri", name="tri")
            nc.scalar.dma_start(tri_t[:], tri[:])
            ones_t = tabs.tile([P, P], F32R, tag="ones", name="ones")
            nc.scalar.dma_start(ones_t[:], onesm[:])
            bvc_t = tabs.tile([P, HPC], F32, tag="bvc", name="bvc")
            nc.scalar.dma_start(bvc_t[:], bvc[:])

            # ---- phase A: V into SBUF (two n-passes over features) ----
            v_tiles = [
                v_res.tile([P, HPC * D], BF16, tag=f"v{j}", name=f"v{j}")
                for j in range(T // P)
            ]
            y_tiles = [
                y_res.tile([P, T], BF16, tag=f"y{h}", name=f"y{h}")
                for h in range(HPC)
            ]
            with tc.tile_pool(name="wv_pool", bufs=1) as wv_pool:
                wvh = [[None] * CC, [None] * CC]
                for c in range(CC):
                    w_ = wv_pool.tile([P, NT], BF16, tag=f"wva{c}",
                                      name=f"wva{c}")
                    nc.sync.dma_start(w_[:], wv[c * P:(c + 1) * P, 0:NT])
                    wvh[0][c] = w_
                xtt = [[None] * TT for _ in range(CC)]

                def load_xt_slice(t):
                    for c in range(CC):
                        x_ = xt_res.tile([P, NT], BF16, tag=f"x{c}_{t}",
                                         name=f"x{c}_{t}")
                        nc.sync.dma_start(
                            x_[:],
                            xt[c * P:(c + 1) * P, t * NT:(t + 1) * NT],
                        )
                        xtt[c][t] = x_

                load_xt_slice(0)
                for c in range(CC):
                    w_ = wv_pool.tile([P, NT], BF16, tag=f"wva{c}",
                                      name=f"wvb{c}")
                    nc.sync.dma_start(w_[:], wv[c * P:(c + 1) * P, NT:2 * NT])
                    wvh[1][c] = w_
                for t in range(1, TT):
                    load_xt_slice(t)

                for n in range(2):
                    for t in range(TT):
                        for m in range(4):
                            mtok = 4 * t + m
                            msl = slice(m * P, (m + 1) * P)
                            ps = psum.tile([P, NT], F32, tag="pst",
                                           bufs=2, name="psa")
                            for c in range(CC):
                                _mm(nc, ps[:], xtt[c][t][:, msl],
                                    wvh[n][c][:],
                                    start=(c == 0), stop=(c == CC - 1))
                            nc.vector.tensor_copy(
                                v_tiles[mtok][:, n * NT:(n + 1) * NT],
                                ps[:])

            # ---- head loop: B_h (qk+rope) then C_h (SDPA) ----
            wq_pool = es.enter_context(tc.tile_pool(name="wq_pool", bufs=1))
            wp_t = []
            for h in range(HPC):
                # B_h
                qt = qk_res.tile([P, T], BF16, tag="qhat", bufs=2,
                                 name="qhat")
                kt = qk_res.tile([P, T], BF16, tag="khat", bufs=2,
                                 name="khat")
                wq_t = []
                for c in range(CC):
                    w_ = wq_pool.tile([P, 2 * P], BF16, tag=f"wq{c}",
                                      bufs=2, name=f"wq{c}")
                    nc.sync.dma_start(
                        w_[:],
                        wqk[c * P:(c + 1) * P, h * 2 * P:(h + 1) * 2 * P],
                    )
                    wq_t.append(w_)
                for t in range(TT):
                    sl = slice(t * NT, (t + 1) * NT)
                    for f in range(2):  # 0 = q, 1 = k
                        feat = 2 * h + f
                        ps = psum.tile([P, NT], F32, tag="pst", bufs=2,
                                       name="pst")
                        for c in range(CC):
                            _mm(nc, ps[:],
                                wq_t[c][:, f * P:(f + 1) * P],
                                xtt[c][t][:],
                                start=(c == 0), stop=(c == CC - 1))
                        raw = work.tile([P, NT], F32, tag="raw", bufs=2,
                                        name="raw")
                        nc.scalar.activation(
                            raw[:], ps[:], AF.Identity,
                            bias=bqk_t[:, feat:feat + 1],
                        )
                        rsw = work.tile([P, NT], F32, tag="rsw", bufs=2,
                                        name="rsw")
                        nc.gpsimd.tensor_copy(rsw[0:hd, :], raw[hd:P, :])
                        nc.gpsimd.tensor_copy(rsw[hd:P, :], raw[0:hd, :])
                        tm1 = work.tile([P, NT], F32, tag="tm1", bufs=2,
                                        name="tm1")
                        tm2 = work.tile([P, NT], F32, tag="tm2", bufs=2,
                                        name="tm2")
                        nc.vector.tensor_mul(tm1[:], raw[:], cs_t[:, sl])
                        nc.vector.tensor_mul(tm2[:], rsw[:], sw_t[:, sl])
                        dest = qt if f == 0 else kt
                        nc.vector.tensor_add(dest[:, sl], tm1[:], tm2[:])

                if h == HPC - 1:
                    # x^T and wq no longer needed; free their SBUF and
                    # stream wp into the freed space during C_7
                    es.close()
                    wp_pool = es2.enter_context(
                        tc.tile_pool(name="wp_pool", bufs=1))
                    for hh in range(HPC):
                        for half in range(2):
                            w_ = wp_pool.tile(
                                [P, C // 2], BF16, tag=f"wp{hh}_{half}",
                                name=f"wp{hh}_{half}")
                            nc.sync.dma_start(
                                w_[:],
                                wp[hh * P:(hh + 1) * P,
                                   half * (C // 2):(half + 1) * (C // 2)],
                            )
                            wp_t.append(w_)

                # C_h: t-outer SDPA
                for t in range(TT):
                    psy = psum.tile([P, NT], F32, tag="psy", bufs=2,
                                    name="psy")
                    p_sum = work.tile([P, NT], F32R, tag="p_sum", bufs=2,
                                      name="p_sum")
                    jmax = 4 * t + 3
                    for j in range(jmax + 1):
                        diag = (j // 4 == t)
                        off = (j % 4) * P if diag else 0
                        pss = psum.tile([P, NT], F32, tag="pss", bufs=3,
                                        name="pss")
                        _mm(nc, pss[:, off:], kt[:, j * P:(j + 1) * P],
                            qt[:, t * NT + off:(t + 1) * NT],
                            start=True, stop=True)
                        p = work.tile([P, NT], BF16, tag="p", bufs=4,
                                      name="p")
                        nc.scalar.activation(p[:, off:], pss[:, off:],
                                             AF.Exp)
                        if diag:
                            nc.vector.tensor_mul(
                                p[:, off:off + P],
                                p[:, off:off + P],
                                tri_t[:],
                            )
                        if j == 0:
                            nc.vector.tensor_copy(p_sum[:], p[:])
                        else:
                            nc.vector.tensor_add(
                                p_sum[:, off:], p_sum[:, off:],
                                p[:, off:])
                        _mm(nc, psy[:, off:],
                            v_tiles[j][:, h * D:(h + 1) * D],
                            p[:, off:],
                            start=(j == 0), stop=(j == jmax))
                    psd = psum.tile([P, NT], F32, tag="psd", bufs=1,
                                    name="psd")
                    _mm(nc, psd[:], ones_t[:], p_sum[:],
                        start=True, stop=True)
                    rden = work.tile([P, NT], F32, tag="rden", bufs=2,
                                     name="rden")
                    nc.vector.reciprocal_approx_fast(rden[:], psd[:])
                    ysl = y_tiles[h][:, t * NT:(t + 1) * NT]
                    nc.vector.tensor_mul(ysl, psy[:], rden[:])
                    nc.vector.tensor_scalar_add(
                        ysl, ysl, scalar1=bvc_t[:, h:h + 1])

            # ---- phase D: projection ----
            with tc.tile_pool(name="ot_pool", bufs=1) as ot_pool:
                for half in range(2):
                    base = half * (C // 2)
                    for m in range(T // P):
                        msl = slice(m * P, (m + 1) * P)
                        pso = [
                            psum.tile([P, NT], F32, tag=t_,
                                      bufs=2, name=f"pso{n}")
                            for n, t_ in enumerate(("pst", "psy"))
                        ]
                        for hh in range(HPC):
                            lhsT = y_tiles[hh][:, msl]
                            for n in range(2):
                                _mm(nc, pso[n][:], lhsT,
                                    wp_t[2 * hh + half][:, n * NT:
                                                        (n + 1) * NT],
                                    start=(hh == 0),
                                    stop=(hh == HPC - 1))
                        ot = ot_pool.tile([P, C // 2], F32, tag="ot",
                                          bufs=3, name="ot")
                        for n in range(2):
                            nc.vector.tensor_copy(
                                ot[:, n * NT:(n + 1) * NT], pso[n][:])
                        nc.scalar.dma_start(
                            out[msl, base:base + C // 2], ot[:])
            es2.close()

    nc.finalize()
    return nc


def prep_inputs(x, w_attn, b_attn, w_proj, b_proj):
    """Build the 8 per-core input maps from full inputs."""
    x = np.asarray(x, dtype=np.float32)
    w_attn = np.asarray(w_attn, dtype=np.float32)
    b_attn = np.asarray(b_attn, dtype=np.float32)
    w_proj = np.asarray(w_proj, dtype=np.float32)

    scale = np.float32(1.0 / np.sqrt(D))

    inv_freq = 1.0 / (ROPE_BASE ** (np.arange(0, D, 2, dtype=np.float32) / D))
    tpos = np.arange(T, dtype=np.float32)
    ang = np.outer(tpos, inv_freq)  # [T, 64]
    cos_t, sin_t = np.cos(ang).T, np.sin(ang).T  # [64, T]
    cs = np.ascontiguousarray(
        np.concatenate([cos_t, cos_t], axis=0)).astype(np.float16)
    sw = np.ascontiguousarray(
        np.concatenate([-sin_t, sin_t], axis=0)).astype(np.float16)

    qq = np.arange(P)
    kk = np.arange(P)[:, None]
    tri = np.ascontiguousarray(
        (qq[None, :] >= kk).astype(BF16NP))  # [128,128] causal triangle

    onesm = np.ones((P, P), dtype=np.float32)

    in_maps = []
    for core in range(8):
        b = core // 2
        hg = core % 2
        heads = range(hg * HPC, (hg + 1) * HPC)
        qcols = np.concatenate([np.arange(h * D, (h + 1) * D) for h in heads])
        kcols = qcols + C
        vcols = qcols + 2 * C

        wq = w_attn[:, qcols] * scale          # [C, 1024]
        wk = w_attn[:, kcols]                  # [C, 1024]
        # interleave per head: [q0, k0, q1, k1, ...]
        wqk_s = np.empty((C, QK_CHUNKS * P), dtype=np.float32)
        bqk_s = np.empty(QK_CHUNKS * P, dtype=np.float32)
        bq = b_attn[qcols] * scale
        bk = b_attn[kcols]
        for hh in range(HPC):
            wqk_s[:, 2 * hh * P:(2 * hh + 1) * P] = \
                wq[:, hh * P:(hh + 1) * P]
            wqk_s[:, (2 * hh + 1) * P:(2 * hh + 2) * P] = \
                wk[:, hh * P:(hh + 1) * P]
            bqk_s[2 * hh * P:(2 * hh + 1) * P] = bq[hh * P:(hh + 1) * P]
            bqk_s[(2 * hh + 1) * P:(2 * hh + 2) * P] = bk[hh * P:(hh + 1) * P]
        wqk_s = np.ascontiguousarray(wqk_s).astype(BF16NP)
        bqk_s = np.ascontiguousarray(bqk_s.reshape(QK_CHUNKS, P).T)
        wv_s = np.ascontiguousarray(w_attn[:, vcols]).astype(BF16NP)
        bv_s = np.ascontiguousarray(
            b_attn[vcols].reshape(HPC, D).T)  # [128, HPC]
        wp_s = np.ascontiguousarray(w_proj[qcols, :]).astype(BF16NP)
        xt_s = np.ascontiguousarray(x[b].T).astype(BF16NP)

        in_maps.append({
            "xt": xt_s, "wqk": wqk_s, "bqk": bqk_s, "wv": wv_s, "bvc": bv_s,
            "cs": cs, "sw": sw, "tri": tri, "onesm": onesm, "wp": wp_s,
        })
    return in_maps


def _get_program():
    if "nc" not in _CACHE:
        _CACHE["nc"] = build_program()
    return _CACHE["nc"]


def _postprocess(outs, b_proj):
    b_proj = np.asarray(b_proj, dtype=np.float32)
    return np.stack(
        [outs[2 * b] + outs[2 * b + 1] + b_proj[None, :] for b in range(B)]
    ).astype(np.float32)


def _run(inputs, trace=False):
    from concourse.bass_utils import run_bass_kernel_spmd

    nc = _get_program()
    in_maps = prep_inputs(
        inputs["x"], inputs["w_attn"], inputs["b_attn"],
        inputs["w_proj"], inputs["b_proj"],
    )
    res = run_bass_kernel_spmd(nc, in_maps, core_ids=list(range(8)),
                               trace=trace)
    full = _postprocess([r["out"] for r in res.results], inputs["b_proj"])
    return full, res


def kernel(**inputs):
    full, _ = _run(inputs, trace=False)
    return full


if __name__ == "__main__":
    _get_program()
    print("built ok")


# revision 10
# speedup vs baseline: 1.2439x; 1.2439x over previous
"""Trainium2 Bass kernel for causal self-attention (B=4, T=2048, C=2048, H=16).

Sharding: 8 cores = 4 batches x 2 head-groups (8 heads each).

v3: fused per-head pipeline with EXPLICIT instruction interleaving.
  A) v = x @ Wv -> SBUF-resident bf16 [16 x (128, 1024)]
  Head loop h: C_h (SDPA) units interleaved with B_{h+1} (qk+rope) chain
  units so the PE always has matmul work while ACT/DVE chew on exp/softmax.
  C_7 interleaves with phase D (projection) units; head-7's matmul is last
  in each D accumulation chain so partial y_7 availability is enough.
Host sums core pairs per batch and adds b_proj.

All matmul operands bf16 (fp32 PSUM accumulate). x^T resident in SBUF
through phase A + all B_h as 64 [128,512] bf16 tiles. p_sum kept in bf16
(DVE 4x perf mode); denominator via ones-matmul; reciprocal_approx_fast.
"""

import contextlib
import sys

import numpy as np

sys.path.insert(0, "/opt/trn_rl_repo")

import ml_dtypes  # noqa: E402

import concourse.bass as bass  # noqa: E402,F401
import concourse.mybir as mybir  # noqa: E402
import concourse.tile as tile  # noqa: E402
from concourse import bacc  # noqa: E402

F32 = mybir.dt.float32
F32R = mybir.dt.float32r
BF16 = mybir.dt.bfloat16
AF = mybir.ActivationFunctionType
BF16NP = ml_dtypes.bfloat16

B, T, C = 4, 2048, 2048
H, D = 16, 128
HPC = 8            # heads per core
P = 128
NT = 512           # matmul moving free dim
TT = T // NT       # 4 token tiles
CC = C // P        # 16 contraction chunks over C
QK_CHUNKS = 2 * HPC
ROPE_BASE = 10000.0

_CACHE = {}


def _mm(nc, out, lhsT, rhs, **kw):
    nc.tensor.matmul(out, lhsT, rhs, **kw)


def build_program():
    nc = bacc.Bacc(name="csa_v3")

    xt = nc.dram_tensor("xt", (C, T), BF16, kind="ExternalInput")
    # wqk columns interleaved per head: head h owns cols [256h, 256h+256)
    # (first 128 = q scaled, next 128 = k)
    wqk = nc.dram_tensor("wqk", (C, QK_CHUNKS * P), BF16, kind="ExternalInput")
    bqk = nc.dram_tensor("bqk", (P, QK_CHUNKS), F32, kind="ExternalInput")
    wv = nc.dram_tensor("wv", (C, HPC * D), BF16, kind="ExternalInput")
    bvc = nc.dram_tensor("bvc", (P, HPC), F32, kind="ExternalInput")
    cs = nc.dram_tensor("cs", (P, T), BF16, kind="ExternalInput")
    sw = nc.dram_tensor("sw", (P, T), BF16, kind="ExternalInput")
    tri = nc.dram_tensor("tri", (P, P), BF16, kind="ExternalInput")
    onesm = nc.dram_tensor("onesm", (P, P), BF16, kind="ExternalInput")
    wp = nc.dram_tensor("wp", (HPC * D, C), BF16, kind="ExternalInput")
    out = nc.dram_tensor("out", (T, C), F32, kind="ExternalOutput")

    hd = D // 2

    with tile.TileContext(nc) as tc:
        es = contextlib.ExitStack()
        es2 = contextlib.ExitStack()
        with (
            tc.tile_pool(name="tabs", bufs=1) as tabs,
            tc.tile_pool(name="v_res", bufs=1) as v_res,
            tc.tile_pool(name="y_res", bufs=1) as y_res,
            tc.tile_pool(name="qk_res", bufs=1) as qk_res,
            tc.tile_pool(name="work", bufs=1) as work,
            tc.tile_pool(name="psum", bufs=1, space="PSUM") as psum,
        ):
            xt_res = es.enter_context(tc.tile_pool(name="xt_res", bufs=1))

            # ---- tables ----
            bqk_t = tabs.tile([P, QK_CHUNKS], F32, tag="bqk", name="bqk")
            nc.scalar.dma_start(bqk_t[:], bqk[:])
            cs_t = tabs.tile([P, T], BF16, tag="cs", name="cs")
            nc.scalar.dma_start(cs_t[:], cs[:])
            sw_t = tabs.tile([P, T], BF16, tag="sw", name="sw")
            nc.scalar.dma_start(sw_t[:], sw[:])
            tri_t = tabs.tile([P, P], BF16, tag="tri", name="tri")
            nc.scalar.dma_start(tri_t[:], tri[:])
            ones_t = tabs.tile([P, P], BF16, tag="ones", name="ones")
            nc.scalar.dma_start(ones_t[:], onesm[:])
            bvc_t = tabs.tile([P, HPC], F32, tag="bvc", name="bvc")
            nc.scalar.dma_start(bvc_t[:], bvc[:])

            v_tiles = [
                v_res.tile([P, HPC * D], BF16, tag=f"v{j}", name=f"v{j}")
                for j in range(T // P)
            ]
            y_tiles = [
                y_res.tile([P, T], BF16, tag=f"y{h}", name=f"y{h}")
                for h in range(HPC)
            ]

            # ---- phase A: V into SBUF (two n-passes over features) ----
            with tc.tile_pool(name="wv_pool", bufs=1) as wv_pool:
                wvh = [[None] * CC, [None] * CC]
                for c in range(CC):
                    w_ = wv_pool.tile([P, NT], BF16, tag=f"wva{c}",
                                      name=f"wva{c}")
                    nc.sync.dma_start(w_[:], wv[c * P:(c + 1) * P, 0:NT])
                    wvh[0][c] = w_
                xtt = [[None] * TT for _ in range(CC)]

                def load_xt_slice(t):
                    for c in range(CC):
                        x_ = xt_res.tile([P, NT], BF16, tag=f"x{c}_{t}",
                                         name=f"x{c}_{t}")
                        nc.sync.dma_start(
                            x_[:],
                            xt[c * P:(c + 1) * P, t * NT:(t + 1) * NT],
                        )
                        xtt[c][t] = x_

                load_xt_slice(0)
                for c in range(CC):
                    w_ = wv_pool.tile([P, NT], BF16, tag=f"wvb{c}",
                                      name=f"wvb{c}")
                    nc.sync.dma_start(w_[:], wv[c * P:(c + 1) * P, NT:2 * NT])
                    wvh[1][c] = w_
                for t in range(1, TT):
                    load_xt_slice(t)

                for n in range(2):
                    for t in range(TT):
                        for m in range(4):
                            mtok = 4 * t + m
                            msl = slice(m * P, (m + 1) * P)
                            ps = psum.tile([P, NT], F32, tag="pst",
                                           bufs=2, name="psa")
                            for c in range(CC):
                                _mm(nc, ps[:], xtt[c][t][:, msl],
                                    wvh[n][c][:],
                                    start=(c == 0), stop=(c == CC - 1))
                            nc.vector.tensor_copy(
                                v_tiles[mtok][:, n * NT:(n + 1) * NT],
                                ps[:])

            wq_pool = es.enter_context(tc.tile_pool(name="wq_pool", bufs=1))

            # ---------- emission helpers ----------
            state = {"wq": {}, "qt": {}, "kt": {}, "wp": [None] * (2 * HPC)}

            def emit_B_dma(h):
                qt = qk_res.tile([P, T], BF16, tag="qhat", bufs=2,
                                 name="qhat")
                kt = qk_res.tile([P, T], BF16, tag="khat", bufs=2,
                                 name="khat")
                state["qt"][h] = qt
                state["kt"][h] = kt
                wq_t = []
                for c in range(CC):
                    w_ = wq_pool.tile([P, 2 * P], BF16, tag=f"wq{c}",
                                      bufs=2, name=f"wq{c}")
                    nc.sync.dma_start(
                        w_[:],
                        wqk[c * P:(c + 1) * P, h * 2 * P:(h + 1) * 2 * P],
                    )
                    wq_t.append(w_)
                state["wq"][h] = wq_t

            def emit_B_unit(h, t, f):
                """One qk chain: 16 MMs + RoPE for (head h, token tile t,
                f=0 q / f=1 k)."""
                wq_t = state["wq"][h]
                sl = slice(t * NT, (t + 1) * NT)
                feat = 2 * h + f
                ps = psum.tile([P, NT], F32, tag="pst", bufs=2, name="pst")
                for c in range(CC):
                    _mm(nc, ps[:], wq_t[c][:, f * P:(f + 1) * P],
                        xtt[c][t][:],
                        start=(c == 0), stop=(c == CC - 1))
                raw = work.tile([P, NT], F32, tag="raw", bufs=2, name="raw")
                nc.scalar.activation(
                    raw[:], ps[:], AF.Identity,
                    bias=bqk_t[:, feat:feat + 1],
                )
                rsw = work.tile([P, NT], F32, tag="rsw", bufs=2, name="rsw")
                nc.scalar.activation(
                    rsw[0:hd, :], ps[hd:P, :], AF.Identity,
                    bias=bqk_t[hd:P, feat:feat + 1],
                )
                nc.scalar.activation(
                    rsw[hd:P, :], ps[0:hd, :], AF.Identity,
                    bias=bqk_t[0:hd, feat:feat + 1],
                )
                tm1 = work.tile([P, NT], F32, tag="tm1", bufs=2, name="tm1")
                tm2 = work.tile([P, NT], F32, tag="tm2", bufs=2, name="tm2")
                nc.vector.tensor_mul(tm1[:], raw[:], cs_t[:, sl])
                nc.vector.tensor_mul(tm2[:], rsw[:], sw_t[:, sl])
                dest = state["qt"][h] if f == 0 else state["kt"][h]
                nc.vector.tensor_add(dest[:, sl], tm1[:], tm2[:])

            cstate = {}

            def emit_C_unit(h, t, j):
                qt, kt = state["qt"][h], state["kt"][h]
                jmax = 4 * t + 3
                if j == 0:
                    cstate["psy"] = psum.tile([P, NT], F32, tag="psy",
                                              bufs=2, name="psy")
                    cstate["p_sum"] = work.tile([P, NT], BF16, tag="p_sum",
                                                bufs=2, name="p_sum")
                psy = cstate["psy"]
                p_sum = cstate["p_sum"]
                diag = (j // 4 == t)
                off = (j % 4) * P if diag else 0
                pss = psum.tile([P, NT], F32, tag="pss", bufs=3, name="pss")
                _mm(nc, pss[:, off:], kt[:, j * P:(j + 1) * P],
                    qt[:, t * NT + off:(t + 1) * NT],
                    start=True, stop=True)
                p = work.tile([P, NT], BF16, tag="p", bufs=3, name="p")
                nc.scalar.activation(p[:, off:], pss[:, off:], AF.Exp)
                if diag:
                    nc.vector.tensor_mul(
                        p[:, off:off + P], p[:, off:off + P], tri_t[:])
                if j == 0:
                    nc.vector.tensor_copy(p_sum[:], p[:])
                else:
                    nc.vector.tensor_add(
                        p_sum[:, off:], p_sum[:, off:], p[:, off:])
                _mm(nc, psy[:, off:],
                    v_tiles[j][:, h * D:(h + 1) * D], p[:, off:],
                    start=(j == 0), stop=(j == jmax))

            def emit_C_tail(h, t):
                psy = cstate["psy"]
                p_sum = cstate["p_sum"]
                psd = psum.tile([P, NT], F32, tag="psd", bufs=1, name="psd")
                _mm(nc, psd[:], ones_t[:], p_sum[:], start=True, stop=True)
                rden = work.tile([P, NT], F32, tag="rden", bufs=1,
                                 name="rden")
                nc.vector.reciprocal_approx_fast(rden[:], psd[:])
                ysl = y_tiles[h][:, t * NT:(t + 1) * NT]
                nc.vector.tensor_mul(ysl, psy[:], rden[:])
                nc.vector.tensor_scalar_add(
                    ysl, ysl, scalar1=bvc_t[:, h:h + 1])

            def emit_wp_dma():
                es.close()   # free x^T + wq SBUF
                wp_pool = es2.enter_context(
                    tc.tile_pool(name="wp_pool", bufs=1))
                state["ot_pool"] = es2.enter_context(
                    tc.tile_pool(name="ot_pool", bufs=1))
                for hh in range(HPC):
                    for half in range(2):
                        w_ = wp_pool.tile(
                            [P, C // 2], BF16, tag=f"wp{hh}_{half}",
                            name=f"wp{hh}_{half}")
                        nc.scalar.dma_start(
                            w_[:],
                            wp[hh * P:(hh + 1) * P,
                               half * (C // 2):(half + 1) * (C // 2)],
                        )
                        state["wp"][2 * hh + half] = w_

            def emit_D_unit(half, m, n):
                """Projection for output block [m*128:(m+1)*128,
                half*1024 + n*512 ...]: 8 accumulating MMs over heads."""
                msl = slice(m * P, (m + 1) * P)
                pso = psum.tile([P, NT], F32, tag="pst", bufs=2, name="pso")
                for hh in range(HPC):
                    _mm(nc, pso[:],
                        y_tiles[hh][:, msl],
                        state["wp"][2 * hh + half][:, n * NT:(n + 1) * NT],
                        start=(hh == 0), stop=(hh == HPC - 1))
                ot = state["ot_pool"].tile([P, NT], F32, tag="ot",
                                           bufs=3, name="ot")
                nc.vector.tensor_copy(ot[:], pso[:])
                nc.scalar.dma_start(
                    out[msl, half * (C // 2) + n * NT:
                        half * (C // 2) + (n + 1) * NT],
                    ot[:])

            # ---------- head loop with explicit interleave ----------
            # B_0 runs standalone (phase A precedes it)
            emit_B_dma(0)
            for t in range(TT):
                for f in range(2):
                    emit_B_unit(0, t, f)

            d_emitted = set()

            for h in range(HPC):
                c_items = []
                for t in range(TT):
                    for j in range(4 * t + 4):
                        c_items.append(("c", t, j))
                    c_items.append(("ct", t, None))
                if h + 1 < HPC:
                    emit_B_dma(h + 1)
                    fill = [("b", t, f) for t in range(TT) for f in range(2)]
                    fi = 0
                    cnt = 0
                    for kind, t, j in c_items:
                        if kind == "c":
                            emit_C_unit(h, t, j)
                        else:
                            emit_C_tail(h, t)
                        cnt += 1
                        if cnt % 5 == 0 and fi < len(fill):
                            _, bt, bf = fill[fi]
                            emit_B_unit(h + 1, bt, bf)
                            fi += 1
                    while fi < len(fill):
                        _, bt, bf = fill[fi]
                        emit_B_unit(h + 1, bt, bf)
                        fi += 1
                else:
                    # h == 7: free x/wq SBUF, stream wp, interleave D units
                    emit_wp_dma()
                    for kind, t, j in c_items:
                        if kind == "c":
                            emit_C_unit(h, t, j)
                        else:
                            emit_C_tail(h, t)
                            # D units whose y_7 token tile is now complete
                            for m in range(4 * t, 4 * t + 4):
                                for n_ in range(2):
                                    half = (m + n_) % 2
                                    emit_D_unit(half, m, n_)
                                    d_emitted.add((half, m, n_))

            # remaining D units
            for m in range(T // P):
                for n_ in range(2):
                    for half in range(2):
                        if (half, m, n_) not in d_emitted:
                            emit_D_unit(half, m, n_)
            es2.close()

    nc.finalize()
    return nc


def prep_inputs(x, w_attn, b_attn, w_proj, b_proj):
    """Build the 8 per-core input maps from full inputs."""
    x = np.asarray(x, dtype=np.float32)
    w_attn = np.asarray(w_attn, dtype=np.float32)
    b_attn = np.asarray(b_attn, dtype=np.float32)
    w_proj = np.asarray(w_proj, dtype=np.float32)

    scale = np.float32(1.0 / np.sqrt(D))

    inv_freq = 1.0 / (ROPE_BASE ** (np.arange(0, D, 2, dtype=np.float32) / D))
    tpos = np.arange(T, dtype=np.float32)
    ang = np.outer(tpos, inv_freq)  # [T, 64]
    cos_t, sin_t = np.cos(ang).T, np.sin(ang).T  # [64, T]
    cs = np.ascontiguousarray(
        np.concatenate([cos_t, cos_t], axis=0)).astype(BF16NP)
    sw = np.ascontiguousarray(
        np.concatenate([-sin_t, sin_t], axis=0)).astype(BF16NP)

    qq = np.arange(P)
    kk = np.arange(P)[:, None]
    tri = np.ascontiguousarray(
        (qq[None, :] >= kk).astype(BF16NP))  # [128,128] causal triangle

    onesm = np.ones((P, P), dtype=BF16NP)

    in_maps = []
    for core in range(8):
        b = core // 2
        hg = core % 2
        heads = range(hg * HPC, (hg + 1) * HPC)
        qcols = np.concatenate([np.arange(h * D, (h + 1) * D) for h in heads])
        kcols = qcols + C
        vcols = qcols + 2 * C

        wq = w_attn[:, qcols] * scale          # [C, 1024]
        wk = w_attn[:, kcols]                  # [C, 1024]
        # interleave per head: [q0, k0, q1, k1, ...]
        wqk_s = np.empty((C, QK_CHUNKS * P), dtype=np.float32)
        bqk_s = np.empty(QK_CHUNKS * P, dtype=np.float32)
        bq = b_attn[qcols] * scale
        bk = b_attn[kcols]
        for hh in range(HPC):
            wqk_s[:, 2 * hh * P:(2 * hh + 1) * P] = \
                wq[:, hh * P:(hh + 1) * P]
            wqk_s[:, (2 * hh + 1) * P:(2 * hh + 2) * P] = \
                wk[:, hh * P:(hh + 1) * P]
            bqk_s[2 * hh * P:(2 * hh + 1) * P] = bq[hh * P:(hh + 1) * P]
            bqk_s[(2 * hh + 1) * P:(2 * hh + 2) * P] = bk[hh * P:(hh + 1) * P]
        wqk_s = np.ascontiguousarray(wqk_s).astype(BF16NP)
        bqk_s = np.ascontiguousarray(bqk_s.reshape(QK_CHUNKS, P).T)
        wv_s = np.ascontiguousarray(w_attn[:, vcols]).astype(BF16NP)
        bv_s = np.ascontiguousarray(
            b_attn[vcols].reshape(HPC, D).T)  # [128, HPC]
        wp_s = np.ascontiguousarray(w_proj[qcols, :]).astype(BF16NP)
        xt_s = np.ascontiguousarray(x[b].T).astype(BF16NP)

        in_maps.append({
            "xt": xt_s, "wqk": wqk_s, "bqk": bqk_s, "wv": wv_s, "bvc": bv_s,
            "cs": cs, "sw": sw, "tri": tri, "onesm": onesm, "wp": wp_s,
        })
    return in_maps


def _get_program():
    if "nc" not in _CACHE:
        _CACHE["nc"] = build_program()
    return _CACHE["nc"]


def _postprocess(outs, b_proj):
    b_proj = np.asarray(b_proj, dtype=np.float32)
    return np.stack(
        [outs[2 * b] + outs[2 * b + 1] + b_proj[None, :] for b in range(B)]
    ).astype(np.float32)


def _run(inputs, trace=False):
    from concourse.bass_utils import run_bass_kernel_spmd

    nc = _get_program()
    in_maps = prep_inputs(
        inputs["x"], inputs["w_attn"], inputs["b_attn"],
        inputs["w_proj"], inputs["b_proj"],
    )
    res = run_bass_kernel_spmd(nc, in_maps, core_ids=list(range(8)),
                               trace=trace)
    full = _postprocess([r["out"] for r in res.results], inputs["b_proj"])
    return full, res


def kernel(**inputs):
    full, _ = _run(inputs, trace=False)
    return full


if __name__ == "__main__":
    _get_program()
    print("built ok")
